# revision 1
# baseline (speedup 1.0000x reference)
"""FM (factorization machine) embedding lookup kernel for 8 Trainium2 cores.

Strategy: data-parallel over the batch (16384 rows -> 2048 per core). The
concatenated first/second-order embedding tables are combined host-side into
one [3.9M, 17] f32 table (col 0 = first_emb, cols 1:17 = second_emb) so each
(batch, field) lookup is a single 68B indirect-DMA gather row. Each core:

  - loads its idx/xv slabs ([128 part, 16*39]) once,
  - for each of 16 batch tiles (128 rows each) issues one indirect DMA
    gathering 128*39 rows of 17 f32,
  - DVE: em = gathered * xv (broadcast over the 17 row cols),
         sfo[c] = sum_f em[:, f, c]   (c=0 -> first-order sum, c=1.. -> s vec)
  - ACT: Square(em[:, :, 1:]) with accum_out -> q = sum_f sum_j e^2
  - finale: out = sfo0 + 0.5*(sum_j s_j^2 - q) + bias, one DMA store.
"""

import os
import numpy as np
from contextlib import ExitStack

import sys
if "/opt/trn_rl_repo" not in sys.path:
    sys.path.insert(0, "/opt/trn_rl_repo")

NUM_FIELDS = 39
FIELD_VOCAB = 100000
EMBED_DIM = 16
BATCH = 16384
NCORES = 8
P = 128
ROW = 1 + EMBED_DIM          # 17 floats per gathered row
BL = BATCH // NCORES         # 2048 rows per core
RPT = BL // P                # 16 batch tiles (rows per partition)
VTOT = NUM_FIELDS * FIELD_VOCAB

_cache = {}


def _build():
    import concourse.bass as bass
    import concourse.tile as tile
    from concourse import bacc, mybir

    f32 = mybir.dt.float32
    i32 = mybir.dt.int32
    X = mybir.AxisListType.X

    nc = bacc.Bacc("TRN2", target_bir_lowering=False, debug=False,
                   num_devices=NCORES)
    table = nc.dram_tensor("table", [VTOT, ROW], f32, kind="ExternalInput")
    idx = nc.dram_tensor("idx", [BL, NUM_FIELDS], i32, kind="ExternalInput")
    xv = nc.dram_tensor("xv", [BL, NUM_FIELDS], f32, kind="ExternalInput")
    bias = nc.dram_tensor("bias", [1, 1], f32, kind="ExternalInput")
    out = nc.dram_tensor("out", [BL], f32, kind="ExternalOutput")

    F, E, R = NUM_FIELDS, EMBED_DIM, ROW

    with tile.TileContext(nc) as tc, ExitStack() as ctx:
        const = ctx.enter_context(tc.tile_pool(name="const", bufs=1))
        gp = ctx.enter_context(tc.tile_pool(name="gather", bufs=4))
        emp = ctx.enter_context(tc.tile_pool(name="em", bufs=3))
        e2p = ctx.enter_context(tc.tile_pool(name="e2", bufs=2))

        idx_sb = const.tile([P, RPT * F], i32)
        nc.sync.dma_start(out=idx_sb[:],
                          in_=idx.ap().rearrange("(p r) f -> p (r f)", p=P))
        xv_sb = const.tile([P, RPT * F], f32)
        nc.sync.dma_start(out=xv_sb[:],
                          in_=xv.ap().rearrange("(p r) f -> p (r f)", p=P))
        bias1 = const.tile([1, 1], f32)
        nc.sync.dma_start(out=bias1[:1, :1], in_=bias.ap())
        biasP = const.tile([P, 1], f32)
        nc.gpsimd.partition_broadcast(biasP[:], bias1[:1, :1])

        # accumulators across batch tiles
        sfo = const.tile([P, RPT * R], f32)   # col r*17+0: fo sum; +1..17: s
        qc = const.tile([P, RPT], f32)        # sum_f sum_j e^2 per tile

        for r in range(RPT):
            g = gp.tile([P, F * R], f32)
            # HW only supports one offset per partition per indirect DMA
            # (multi-offset lowering is broken in the compiler), so issue
            # one gather per field.
            for f in range(F):
                nc.gpsimd.indirect_dma_start(
                    out=g[:, f * R:(f + 1) * R],
                    out_offset=None,
                    in_=table.ap(),
                    in_offset=bass.IndirectOffsetOnAxis(
                        ap=idx_sb[:, r * F + f:r * F + f + 1], axis=0),
                )
            em = emp.tile([P, F * R], f32)
            nc.vector.tensor_tensor(
                out=em[:].rearrange("p (f c) -> p f c", c=R),
                in0=g[:].rearrange("p (f c) -> p f c", c=R),
                in1=xv_sb[:, r * F:(r + 1) * F].to_broadcast([P, F, R]),
                op=mybir.AluOpType.mult,
            )
            nc.vector.reduce_sum(
                out=sfo[:, r * R:(r + 1) * R],
                in_=em[:].rearrange("p (f c) -> p c f", c=R),
                axis=X,
            )
            e2 = e2p.tile([P, F * E], f32)
            nc.scalar.activation(
                out=e2[:].rearrange("p (f c) -> p f c", c=E),
                in_=em[:].rearrange("p (f c) -> p f c", c=R)[:, :, 1:],
                func=mybir.ActivationFunctionType.Square,
                accum_out=qc[:, r:r + 1],
            )

        # finale: res = sfo0 + 0.5*(sum_j s^2 - q) + bias
        s2 = const.tile([P, RPT * E], f32)
        nc.scalar.activation(
            out=s2[:].rearrange("p (r c) -> p r c", c=E),
            in_=sfo[:].rearrange("p (r c) -> p r c", c=R)[:, :, 1:],
            func=mybir.ActivationFunctionType.Square,
        )
        s2s = const.tile([P, RPT], f32)
        nc.vector.reduce_sum(
            out=s2s[:], in_=s2[:].rearrange("p (r c) -> p r c", c=E), axis=X)
        d = const.tile([P, RPT], f32)
        nc.vector.tensor_tensor(out=d[:], in0=s2s[:], in1=qc[:],
                                op=mybir.AluOpType.subtract)
        hb = const.tile([P, RPT], f32)
        nc.scalar.activation(
            out=hb[:], in_=d[:],
            func=mybir.ActivationFunctionType.Identity,
            scale=0.5, bias=biasP[:, :1])
        res = const.tile([P, RPT], f32)
        nc.vector.tensor_tensor(
            out=res[:], in0=hb[:],
            in1=sfo[:].rearrange("p (r c) -> p r c", c=R)[:, :, 0],
            op=mybir.AluOpType.add)
        nc.sync.dma_start(out=out.ap().rearrange("(p r) -> p r", p=P),
                          in_=res[:])

    nc.compile()
    return nc


def _get_nc():
    if "nc" not in _cache:
        _cache["nc"] = _build()
    return _cache["nc"]


def _prep_inputs(Xi, Xv, first_emb, second_emb, bias):
    table = np.concatenate(
        [np.ascontiguousarray(first_emb, dtype=np.float32),
         np.ascontiguousarray(second_emb, dtype=np.float32)], axis=1)
    offs = (np.arange(NUM_FIELDS, dtype=np.int64) * FIELD_VOCAB)[None, :]
    flat = (np.asarray(Xi, dtype=np.int64) + offs).astype(np.int32)
    xvf = np.ascontiguousarray(Xv, dtype=np.float32)
    b = np.asarray(bias, dtype=np.float32).reshape(1, 1)
    in_maps = []
    for c in range(NCORES):
        sl = slice(c * BL, (c + 1) * BL)
        in_maps.append({
            "table": table,
            "idx": np.ascontiguousarray(flat[sl]),
            "xv": np.ascontiguousarray(xvf[sl]),
            "bias": b,
        })
    return in_maps


def kernel(Xi, Xv, first_emb, second_emb, bias, _trace=False):
    from concourse.bass_utils import run_bass_kernel_spmd
    if _trace:
        import ntff_shim
        ntff_shim.install()

    nc = _get_nc()
    in_maps = _prep_inputs(Xi, Xv, first_emb, second_emb, bias)
    r = run_bass_kernel_spmd(nc, in_maps, list(range(NCORES)), trace=_trace)
    out = np.concatenate([r.results[c]["out"] for c in range(NCORES)])
    if _trace:
        _cache["last_results"] = r
    return out.astype(np.float32)

